# revision 2
# baseline (speedup 1.0000x reference)
# SAGAN self-attention (B=4, H=W=64, C=64, D=8) on 8 TRN2 NeuronCores.
#
# Sharding: core i = (batch b=i//2, half h=i%2); each core computes rows
# [h*2048,(h+1)*2048) of the 4096x4096 attention for its batch, fully fused
# in SBUF (no NxN matrix touches HBM, no collectives).
#
# Structure (v2; 93.4us -> ~70us):
# - Scores are a pure K=64 channel contraction s^T = x^T_chunk . (P x_q^T),
#   P = Wf Wg^T. The per-key softmax bias d_m = bg.(f_m+bf) is folded into
#   the PV stationary as exp(d_m)*[hv | 1] (exp(s+d) = exp(s) exp(d)), so
#   the exp path needs no bias and K stays exactly 64.
# - K=64 enables ROW-TILED score pairs: chunk 2j occupies PE rows 0:64 and
#   chunk 2j+1 rows 64:128 (tile_position (0,0)/(64,0)); one 512-column
#   stream produces both chunks (2x score throughput, measured 216ns/pair).
#   Both strips together keep all 128 PE rows active, which the HAM clock
#   gate counts as busy.
# - PV is 4-way COL-TILED: the stationary [hv(8)|e^d(1)] is 9 wide, so
#   chunk c accumulates in col strip c%4 at tile_position (0,32s); the
#   4 partial (v_un,den) strips in one PSUM bank are summed for free by the
#   epilogue's 128-partition contraction (Wv/E8 replicated per strip).
# - exp SPLITS across two engines: ScalarE exp LUT on 9/16 pairs (FD=1024
#   calls) and the otherwise-idle VectorE on 7/16 pairs via a Schraudolph
#   bit-trick exp: int16(s*128/ln2 + 16248.5) bitcast to bf16 ~= e^s
#   (+-3%, exact softmax normalization cancels the common mode). The DVE
#   side runs chunk-granular (FD=512) through a 3-slot PSUM ring so the
#   DVE streams back-to-back instead of a sem-bound pair round-trip.
# - PVs are issued three pairs late so the in-order PE queue never stalls
#   on an exp in flight; the previous n-tile epilogue is issued early in
#   the next n-tile for the same reason.
# - The HAM clock gate re-throttles the PE (2.4 -> 1.2 GHz) if PE duty
#   drops too low, and only fully-dense streams re-warm it, so the kernel
#   front-loads a dense warmup during the input DMA wait and sprinkles
#   small dummy MMs (never read) to hold duty above the threshold.
# - psv and pse share one rotating PSUM bank: pse is live only between the
#   vd2 cast and the STT reads, exactly when psv is idle; stale values in
#   rows outside the PV strips are finite and zero-masked by WV2/E8 rows.
#
# PSUM budget: pa 2x[128,1024](4 banks) + pd 3x[128,512](3) +
# psv/pse shared rotating bank (1) = 8 banks.
#
# Host precomputes the tiny projections (0.6% of FLOPs) and folds gamma
# into Wv and gamma*(bh@Wv+bv) into the residual; attention matmuls use
# bf16 operands, PSUM accumulation is fp32.

import numpy as np
import ml_dtypes

import concourse.bacc as bacc
import concourse.tile as tile
import concourse.mybir as mybir
from concourse.alu_op_type import AluOpType
from concourse.bass_utils import run_bass_kernel_spmd

F32 = mybir.dt.float32
BF16 = mybir.dt.bfloat16
I16 = mybir.dt.int16
AFT = mybir.ActivationFunctionType

B, HH, WW, C = 4, 64, 64, 64
D = 8
N = HH * WW          # 4096 sequence positions per batch
RPC = N // 2         # rows (queries) per core
NCORES = 8
TN = 512             # queries per n-tile
NT = RPC // TN       # 4 n-tiles
NPAIR = 16           # key-chunk pairs per n-tile (32 chunks of 128 keys)
MC = 32
SCH_A = 128.0 / np.log(2.0)   # Schraudolph scale for bf16 bit pattern
SCH_B = 16256.0 - 7.5         # 127*128 with centering correction
# exp-lane per pair: A = ScalarE exp LUT, D = DVE Schraudolph
LANES = "ADADADADADADADAA"


def _build():
    nc = bacc.Bacc("TRN2", target_bir_lowering=False, debug=False,
                   num_devices=NCORES)

    xt2p = nc.dram_tensor("xt2p", [128, 2048], BF16, kind="ExternalInput").ap()
    gp2 = nc.dram_tensor("gp2", [128, RPC], BF16, kind="ExternalInput").ap()
    hvo = nc.dram_tensor("hvo", [128, MC * 9], BF16,
                         kind="ExternalInput").ap()
    xrp = nc.dram_tensor("xrp", [128, RPC // 128 * C], F32,
                         kind="ExternalInput").ap()
    wv2 = nc.dram_tensor("wv2", [128, C], BF16, kind="ExternalInput").ap()
    e8 = nc.dram_tensor("e8", [128, 1], BF16, kind="ExternalInput").ap()
    out = nc.dram_tensor("out", [RPC, C], F32, kind="ExternalOutput").ap()

    with tile.TileContext(nc) as tc:
        with tc.tile_pool(name="const", bufs=1) as const:
            XT2P = const.tile([128, 2048], BF16)
            GP2 = const.tile([128, RPC], BF16)
            HVO = const.tile([128, MC * 9], BF16)
            XRP = const.tile([128, RPC // 128 * C], F32)
            WV2 = const.tile([128, C], BF16)
            E8 = const.tile([128, 1], BF16)
            PRE = const.tile([1, 2], F32)
            WUP = const.tile([128, 512], BF16)
            # input DMAs in first-use order
            nc.sync.dma_start(E8[:], e8[:])
            nc.sync.dma_start(GP2[:, 0:512], gp2[:, 0:512])
            nc.sync.dma_start(XT2P[:, 0:256], xt2p[:, 0:256])
            nc.sync.dma_start(HVO[:], hvo[:])
            for j in range(1, 8):
                nc.sync.dma_start(XT2P[:, j * 256:(j + 1) * 256],
                                  xt2p[:, j * 256:(j + 1) * 256])
            for j in range(1, 4):
                nc.sync.dma_start(GP2[:, j * 512:(j + 1) * 512],
                                  gp2[:, j * 512:(j + 1) * 512])
            nc.sync.dma_start(WV2[:], wv2[:])
            nc.sync.dma_start(XRP[:], xrp[:])
            nc.vector.memset(WUP[:], 0.0)
            # hoist the one-time ACT exp table load (~2.7us) into DMA wait
            nc.scalar.activation(PRE[:, 0:1], E8[0:1, :], AFT.Exp)

            with tc.tile_pool(name="pa", bufs=2, space="PSUM") as pap, \
                 tc.tile_pool(name="pd", bufs=3, space="PSUM") as pdp, \
                 tc.tile_pool(name="pv", bufs=1, space="PSUM") as pvp, \
                 tc.tile_pool(name="exa", bufs=5) as exap, \
                 tc.tile_pool(name="exd", bufs=6) as exdp, \
                 tc.tile_pool(name="vd2p", bufs=2) as vd2p, \
                 tc.tile_pool(name="scolp", bufs=2) as scolp, \
                 tc.tile_pool(name="osbp", bufs=2) as osbp:
                # psv and pse share one rotating PSUM bank (pv pool, bufs=1):
                # pse is only live between the vd2 cast and the STT reads,
                # exactly when psv is idle. Stale pse values left in psv rows
                # outside the PV strips are finite and zero-masked by WV2/E8.
                # PE warm-up during the input DMA wait: lifts the HAM clock
                # throttle (1.2 -> 2.4 GHz) before the pipeline begins; also
                # zeroes cols 0:256 of the shared bank
                # 16 x FD=512 cold MMs span ~6.8us: longer than one full
                # free-running 3.4us HAM window, so the un-throttle fires
                # deterministically regardless of window phase
                wps = pvp.tile([128, TN], F32, tag="pp")
                for wi in range(16):
                    nc.tensor.matmul(wps[:], lhsT=WUP[:, 0:128],
                                     rhs=WUP[:], start=True, stop=True,
                                     skip_group_check=True)
                nc.vector.memset(wps[:], 0.0)

                def epilogue(nt, vd2, pse):
                    scol = scolp.tile([128, 4], F32)
                    for nb in range(4):
                        nc.tensor.matmul(pse[:, 256 + nb:257 + nb],
                                         lhsT=vd2[:, nb * 128:(nb + 1) * 128],
                                         rhs=E8[:], start=True, stop=True)
                    for nb in range(4):
                        nc.tensor.matmul(pse[:, nb * 64:(nb + 1) * 64],
                                         lhsT=vd2[:, nb * 128:(nb + 1) * 128],
                                         rhs=WV2[:], start=True, stop=True)
                    nc.vector.reciprocal(scol[:], pse[:, 256:260])
                    osb4 = osbp.tile([128, 4 * C], F32)
                    for nb in range(4):
                        t = nt * 4 + nb
                        nc.vector.scalar_tensor_tensor(
                            osb4[:, nb * C:(nb + 1) * C],
                            pse[:, nb * 64:(nb + 1) * 64],
                            scol[:, nb:nb + 1],
                            XRP[:, t * C:(t + 1) * C],
                            op0=AluOpType.mult, op1=AluOpType.add)
                    dst = out[nt * 512:(nt + 1) * 512, :].rearrange(
                        "(t p) c -> p t c", p=128)
                    nc.sync.dma_start(dst,
                                      osb4[:].rearrange("p (t c) -> p t c",
                                                        c=C))

                def emit_pv(psv, j, exs):
                    for k in range(2):
                        c = 2 * j + k
                        s = c % 4
                        nc.tensor.matmul(psv[32 * s:32 * s + 9, :],
                                         lhsT=HVO[:, 9 * c:9 * c + 9],
                                         rhs=exs[k],
                                         start=(c < 4), stop=(c >= 28),
                                         tile_position=(0, 32 * s),
                                         skip_group_check=True)

                pending = None
                for nt in range(NT):
                    n0 = nt * TN
                    psv = pvp.tile([128, TN], F32, tag="pp")
                    pvq = []   # (j, exs) awaiting PV issue, three pairs late
                    dumq = []  # A-pair score tiles for HAM-filler dummy MMs
                    for j in range(NPAIR):
                        if LANES[j] == "A":
                            ps = pap.tile([128, 1024], F32)
                            ex = exap.tile([128, 1024], BF16)
                            nc.tensor.matmul(ps[:, 0:512],
                                             lhsT=XT2P[0:64,
                                                       j * 128:(j + 1) * 128],
                                             rhs=GP2[0:64, n0:n0 + TN],
                                             start=True, stop=True,
                                             tile_position=(0, 0),
                                             skip_group_check=True)
                            nc.tensor.matmul(ps[:, 512:1024],
                                             lhsT=XT2P[64:128,
                                                       j * 128:(j + 1) * 128],
                                             rhs=GP2[64:128, n0:n0 + TN],
                                             start=True, stop=True,
                                             tile_position=(64, 0),
                                             skip_group_check=True)
                            nc.scalar.activation(ex[:], ps[:], AFT.Exp)
                            pvq.append((j, (ex[:, 0:512], ex[:, 512:1024])))
                            dumq.append(ps)
                        else:
                            # chunk-granular D-lane: 3-slot PSUM ring keeps
                            # the DVE streaming back-to-back instead of the
                            # sem-bound pair round-trip
                            p0 = pdp.tile([128, 512], F32, tag="pd")
                            p1 = pdp.tile([128, 512], F32, tag="pd")
                            e0 = exdp.tile([128, 512], BF16, tag="exd")
                            e1 = exdp.tile([128, 512], BF16, tag="exd")
                            nc.tensor.matmul(p0[:],
                                             lhsT=XT2P[0:64,
                                                       j * 128:(j + 1) * 128],
                                             rhs=GP2[0:64, n0:n0 + TN],
                                             start=True, stop=True,
                                             tile_position=(0, 0),
                                             skip_group_check=True)
                            nc.tensor.matmul(p1[:],
                                             lhsT=XT2P[64:128,
                                                       j * 128:(j + 1) * 128],
                                             rhs=GP2[64:128, n0:n0 + TN],
                                             start=True, stop=True,
                                             tile_position=(64, 0),
                                             skip_group_check=True)
                            nc.vector.tensor_scalar(
                                e0[:].bitcast(I16), p0[:], SCH_A, SCH_B,
                                op0=AluOpType.mult, op1=AluOpType.add)
                            nc.vector.tensor_scalar(
                                e1[:].bitcast(I16), p1[:], SCH_A, SCH_B,
                                op0=AluOpType.mult, op1=AluOpType.add)
                            pvq.append((j, (e0[:], e1[:])))
                        # PVs issue three pairs late: the in-order PE queue
                        # then never stalls on an exp still in flight
                        if len(pvq) > 3:
                            jj, exs = pvq.pop(0)
                            emit_pv(psv, jj, exs)
                            # HAM filler: a dummy MM over the already-read
                            # A score tile keeps PE duty above the throttle
                            # hold threshold; result is never read
                            if dumq and LANES[jj] == "A":
                                dps = dumq.pop(0)
                                dum_n = getattr(emit_pv, "_dn", 0)
                                emit_pv._dn = dum_n + 1
                                if dum_n % 2 == 0:
                                    nc.tensor.matmul(dps[:, 0:256],
                                                     lhsT=WUP[:, 0:128],
                                                     rhs=WUP[:, 0:256],
                                                     start=True, stop=True,
                                                     skip_group_check=True)
                        # previous n-tile's epilogue lands in this n-tile's
                        # early pipeline so the PE never stalls on it
                        if j == 1 and pending is not None:
                            epilogue(*pending)
                            pending = None
                    for jj, exs in pvq:
                        emit_pv(psv, jj, exs)
                    vd2 = vd2p.tile([128, TN], BF16)
                    nc.scalar.copy(vd2[:], psv[:])
                    # pse allocated here so the shared-bank rotation order is
                    # ... psv(nt) -> pse(nt) -> psv(nt+1) ...
                    pse = pvp.tile([128, TN], F32, tag="pp")
                    pending = (nt, vd2, pse)
                epilogue(*pending)
    nc.compile()
    return nc


_CACHE = {}


def _get_compiled():
    if "nc" not in _CACHE:
        _CACHE["nc"] = _build()
    return _CACHE["nc"]


def _make_in_maps(x, Wf, bf, Wg, bg, Wh, bh, Wv, bv, gamma):
    x = np.asarray(x, np.float32)
    Wf = np.asarray(Wf, np.float32)
    Wg = np.asarray(Wg, np.float32)
    Wh = np.asarray(Wh, np.float32)
    Wv = np.asarray(Wv, np.float32)
    bf = np.asarray(bf, np.float32)
    bg = np.asarray(bg, np.float32)
    bh = np.asarray(bh, np.float32)
    bv = np.asarray(bv, np.float32)
    g0 = float(np.asarray(gamma, np.float32).reshape(-1)[0])

    xf = x.reshape(B, N, C)
    P = Wf @ Wg.T                            # [C, C] score kernel
    wfbg = Wf @ bg                           # [C] per-key bias direction
    bgbf = float(bg @ bf)
    res_bias = g0 * (bh @ Wv + bv)           # [C] folded into residual
    wv2 = np.zeros((128, C), np.float32)
    for s4 in range(4):
        wv2[32 * s4:32 * s4 + D] = g0 * Wv
    wv2 = wv2.astype(ml_dtypes.bfloat16)
    e8 = np.zeros((128, 1), np.float32)
    e8[[32 * s4 + 8 for s4 in range(4)]] = 1.0
    e8 = e8.astype(ml_dtypes.bfloat16)

    in_maps = []
    for i in range(NCORES):
        b, h = divmod(i, 2)
        r0 = h * RPC
        xt = xf[b].T                         # [C, N]
        # row-tiled pair layout: rows 0:64 = chunk 2j keys, 64:128 = 2j+1
        xt2p = xt.reshape(C, NPAIR, 2, 128).transpose(2, 0, 1, 3) \
                 .reshape(128, 2048)
        gp_half = P @ xf[b, r0:r0 + RPC].T   # [C, RPC]
        gp2 = np.concatenate([gp_half, gp_half], axis=0)  # duplicate halves
        d = xf[b] @ wfbg + bgbf              # [N] per-key softmax bias
        ed = np.exp(d)
        hv = xf[b] @ Wh                      # [N, D] (bh folds into res_bias)
        ho = np.concatenate([hv * ed[:, None], ed[:, None]], axis=1)
        hvo = np.ascontiguousarray(
            ho.reshape(MC, 128, 9).transpose(1, 0, 2).reshape(128, MC * 9))
        xr = xf[b, r0:r0 + RPC] + res_bias   # [RPC, C]
        xrp = np.ascontiguousarray(
            xr.reshape(RPC // 128, 128, C).transpose(1, 0, 2).reshape(128, -1))
        in_maps.append({"xt2p": np.ascontiguousarray(xt2p)
                        .astype(ml_dtypes.bfloat16),
                        "gp2": np.ascontiguousarray(gp2)
                        .astype(ml_dtypes.bfloat16),
                        "hvo": hvo.astype(ml_dtypes.bfloat16),
                        "xrp": xrp, "wv2": wv2, "e8": e8})
    return in_maps


def _assemble(results):
    outf = np.empty((B, N, C), np.float32)
    for i in range(NCORES):
        b, h = divmod(i, 2)
        outf[b, h * RPC:(h + 1) * RPC] = results[i]["out"]
    return outf.reshape(B, HH, WW, C)


def run(inputs, **spmd_kwargs):
    """Returns (output, BassKernelResults)."""
    nc = _get_compiled()
    in_maps = _make_in_maps(**inputs)
    res = run_bass_kernel_spmd(nc, in_maps, core_ids=list(range(NCORES)),
                               **spmd_kwargs)
    return _assemble(res.results), res


def kernel(**inputs):
    out, _ = run(inputs)
    return out
